# revision 23
# baseline (speedup 1.0000x reference)
"""Batch-invariant linear (out = x @ W.T + b) on 8 TRN2 NeuronCores.

Strategy: data-parallel over the 8192 (batch*seq) rows — 1024 rows/core.
Per core we compute out^T[n, m] so the contraction dim K lands on SBUF
partitions for both operands with no on-device transposes:
  - host pre-transposes x -> xT [K, M] and packs W into per-n-strip
    contiguous blocks [NT, 128, KT*128] (layout prep only),
  - stationary operand = WT tile [128k, 128n], moving = xT [128k, 512m],
  - f32r (TF32) matmuls at full PE rate, fp32 PSUM accumulation over the
    32 k-tiles in increasing k order (deterministic, batch-invariant),
  - startup: 4 n-strips accumulate k-major (filling all 8 PSUM banks)
    so the PE stays busy while the 16.8 MB x shard streams in,
  - bias added on ScalarE during PSUM->SBUF drain, DMA out^T shard out.
Host gathers the 8 out^T shards and transposes back.
"""

import numpy as np

N_CORES = 8
B, S, K, N = 4, 2048, 4096, 4096
M_TOTAL = B * S              # 8192 rows
M = M_TOTAL // N_CORES       # 1024 rows per core
P = 128                      # partitions
KT = K // P                  # 32 k-tiles
NT = N // P                  # 32 n-tiles (out^T partition tiles)
MC = 512                     # moving chunk (max free dim for 4-byte matmul)
PHA = 4                      # n-strips accumulated concurrently at startup

_cache = {}


def _build_nc(Kd=K, Nd=N, Md=M, compute_dt_name="float32r"):
    import concourse.bacc as bacc
    import concourse.mybir as mybir
    import concourse.tile as tile

    kt_n = Kd // P
    nt_n = Nd // P
    nmc = Md // MC
    pha = min(PHA, nt_n)
    kh = max(kt_n // 4, 1)       # k-tiles per W sub-tile
    nwh = (kt_n + kh - 1) // kh  # sub-tiles per strip
    n_oc = 4                     # drain chunks per strip

    cdt = getattr(mybir.dt, compute_dt_name)
    f32 = mybir.dt.float32

    nc = bacc.Bacc("TRN2", target_bir_lowering=False, debug=False)

    xt_d = nc.dram_tensor("xt", [Kd, Md], cdt, kind="ExternalInput").ap()
    wt_d = nc.dram_tensor("wt", [nt_n, P, kt_n * P], cdt,
                          kind="ExternalInput").ap()
    bt_d = nc.dram_tensor("bt", [P, nt_n], f32, kind="ExternalInput").ap()
    ot_d = nc.dram_tensor("ot", [Nd, Md], f32, kind="ExternalOutput").ap()

    with tile.TileContext(nc) as tc:
        with (
            tc.tile_pool(name="xpool", bufs=kt_n) as xpool,
            tc.tile_pool(name="wpool", bufs=min(4 * nwh, nwh * nt_n))
                as wpool,
            tc.tile_pool(name="psum", bufs=min(4, nt_n),
                         space="PSUM") as psumpool,
            tc.tile_pool(name="opool", bufs=4) as opool,
            tc.tile_pool(name="bpool", bufs=1) as bpool,
        ):
            w_tiles = {}   # (nt, half) -> tile

            def load_wh(nt, h):
                w_sb = wpool.tile([P, kh * P], cdt, tag="w",
                                  name=f"w{nt}_{h}")
                nc.sync.dma_start(
                    w_sb[:], wt_d[nt][:, h * kh * P:(h + 1) * kh * P])
                w_tiles[(nt, h)] = w_sb

            def mm(ps, nt, kt, mc):
                w_sb = w_tiles[(nt, kt // kh)]
                nc.tensor.matmul(
                    ps[:, mc * MC:(mc + 1) * MC],
                    w_sb[:, (kt % kh) * P:(kt % kh + 1) * P],
                    x_tiles[kt][:, mc * MC:(mc + 1) * MC],
                    start=(kt == 0),
                    stop=(kt == kt_n - 1),
                )

            def drain(nt, ps, chunks=n_oc, dma_engine=None):
                # chunked, alternating ScalarE/VectorE so the PSUM drain is
                # 2x wide; out DMA off the critical queues
                dma_engine = dma_engine or nc.gpsimd
                h = Md // chunks
                for i in range(chunks):
                    sl = slice(i * h, (i + 1) * h)
                    out_sb = opool.tile([P, h], f32, tag="o")
                    if i % 2 == 0:
                        nc.scalar.activation(
                            out_sb[:], ps[:, sl],
                            mybir.ActivationFunctionType.Identity,
                            bias=bias_sb[:, nt:nt + 1],
                        )
                    else:
                        nc.vector.tensor_scalar_add(
                            out_sb[:], ps[:, sl], bias_sb[:, nt:nt + 1])
                    dma_engine.dma_start(ot_d[nt * P:(nt + 1) * P, sl],
                                         out_sb[:])

            def load_w_strip(nt):
                for q in range(nwh):
                    load_wh(nt, q)

            def release_w(nt):
                for q in range(nwh):
                    del w_tiles[(nt, q)]

            # Startup issue order: interleave the phase-A W halves with the
            # first x k-tiles so the PE can start at the first (w,x) pair.
            x_tiles = []

            def load_next_x(n=1):
                for _ in range(n):
                    kt = len(x_tiles)
                    if kt >= kt_n:
                        return
                    x_sb = xpool.tile([P, Md], cdt, tag="x", name=f"x{kt}")
                    nc.sync.dma_start(x_sb[:], xt_d[kt * P:(kt + 1) * P, :])
                    x_tiles.append(x_sb)

            # PE warm-up: dummy matmuls on zeroed scratch un-throttle the
            # HAM clock gate (~3.4us of sustained activity) while the first
            # DMAs are still in flight, so real matmuls start at 2.4 GHz.
            warm_sb = bpool.tile([P, 256], f32, name="warm")
            nc.vector.memset(warm_sb[:], 0.0)
            warm_ps = psumpool.tile([P, 256], f32, tag="ps", name="warmps")
            for _ in range(10):
                nc.tensor.matmul(warm_ps[:], warm_sb[:, 0:P], warm_sb[:],
                                 start=True, stop=True)

            # Issue order on the sync queue follows phase A's need order:
            # w_s half-0 just before the x tiles strip s will chew first.
            load_wh(0, 0)
            load_next_x(2)
            for s in range(1, pha):
                load_wh(s, 0)
                load_next_x(2)
            bias_sb = bpool.tile([P, nt_n], f32)
            nc.sync.dma_start(bias_sb[:], bt_d[:])
            load_next_x(4)
            for q in range(1, nwh):
                for s in range(pha):
                    load_wh(s, q)
                    load_next_x(1)
            load_next_x(kt_n)

            # Phase A: strips 0..pha-1 accumulate while x streams. Walk
            # k-blocks with the strip loop outside the block's k-loop so the
            # first strip only needs its own W half plus the first x tiles.
            pss = [psumpool.tile([P, Md], f32, tag="ps", name=f"ps{s}")
                   for s in range(pha)]
            kb_sz = min(8, kt_n)
            for kb in range(0, kt_n, kb_sz):
                for s in range(pha):
                    for kt in range(kb, kb + kb_sz):
                        for mc in range(nmc):
                            mm(pss[s], s, kt, mc)
            # Prefetch the next W strips as slots free up.
            for nt in range(pha, min(pha + 2, nt_n)):
                load_w_strip(nt)
            for s in range(pha):
                drain(s, pss[s])
                release_w(s)

            # Phase B: one strip at a time.
            for nt in range(pha, nt_n):
                if nt + 2 < nt_n:
                    load_w_strip(nt + 2)
                ps = psumpool.tile([P, Md], f32, tag="ps")
                for kt in range(kt_n):
                    for mc in range(nmc):
                        mm(ps, nt, kt, mc)
                if nt == nt_n - 1:
                    # final strip: fine chunks on the idle sync queue to
                    # shorten the exposed tail
                    drain(nt, ps, chunks=8, dma_engine=nc.sync)
                else:
                    drain(nt, ps)
                release_w(nt)

    nc.compile()
    return nc


def _get_nc():
    if "nc" not in _cache:
        _cache["nc"] = _build_nc()
    return _cache["nc"]


def _pack_w(weight, Nd=N, Kd=K):
    nt_n, kt_n = Nd // P, Kd // P
    # packed[nt, p, kt, nl] = weight[nt*P + nl, kt*P + p]
    wr = weight.reshape(nt_n, P, kt_n, P)          # [nt, nl, kt, p]
    return np.ascontiguousarray(
        wr.transpose(0, 3, 2, 1)).reshape(nt_n, P, kt_n * P)


def _prep_inputs(x, weight, b):
    x = np.ascontiguousarray(x, dtype=np.float32)
    weight = np.ascontiguousarray(weight, dtype=np.float32)
    b = np.ascontiguousarray(b, dtype=np.float32)

    xt = np.ascontiguousarray(x.reshape(M_TOTAL, K).T)       # [K, M_TOTAL]
    wt = _pack_w(weight)                                     # [NT, P, KT*P]
    bt = np.ascontiguousarray(b.reshape(NT, P).T)            # [P, NT]

    in_maps = []
    for c in range(N_CORES):
        in_maps.append({
            "xt": np.ascontiguousarray(xt[:, c * M:(c + 1) * M]),
            "wt": wt,
            "bt": bt,
        })
    return in_maps


def run(x, weight, b, trace=False, **trace_kwargs):
    from concourse.bass_utils import run_bass_kernel_spmd

    nc = _get_nc()
    in_maps = _prep_inputs(x, weight, b)
    res = run_bass_kernel_spmd(
        nc, in_maps, list(range(N_CORES)), trace=trace, **trace_kwargs
    )

    out = np.empty((M_TOTAL, N), dtype=np.float32)
    for c in range(N_CORES):
        out[c * M:(c + 1) * M, :] = res.results[c]["ot"].T
    return out.reshape(B, S, N), res


def kernel(x, weight, b, tile_size=None):
    out, _ = run(x, weight, b)
    return out


# revision 25
# speedup vs baseline: 1.0053x; 1.0053x over previous
"""Batch-invariant linear (out = x @ W.T + b) on 8 TRN2 NeuronCores.

Strategy: data-parallel over the 8192 (batch*seq) rows — 1024 rows/core.
Per core we compute out^T[n, m] so the contraction dim K lands on SBUF
partitions for both operands with no on-device transposes:
  - host pre-transposes x -> xT [K, M] and packs W into per-n-strip
    contiguous blocks [NT, 128, KT*128] (layout prep only),
  - stationary operand = WT tile [128k, 128n], moving = xT [128k, 512m],
  - f32r (TF32) matmuls at full PE rate, fp32 PSUM accumulation over the
    32 k-tiles in increasing k order (deterministic, batch-invariant),
  - startup: 4 n-strips accumulate k-major (filling all 8 PSUM banks)
    so the PE stays busy while the 16.8 MB x shard streams in,
  - bias added on ScalarE during PSUM->SBUF drain, DMA out^T shard out.
Host gathers the 8 out^T shards and transposes back.
"""

import numpy as np

N_CORES = 8
B, S, K, N = 4, 2048, 4096, 4096
M_TOTAL = B * S              # 8192 rows
M = M_TOTAL // N_CORES       # 1024 rows per core
P = 128                      # partitions
KT = K // P                  # 32 k-tiles
NT = N // P                  # 32 n-tiles (out^T partition tiles)
MC = 512                     # moving chunk (max free dim for 4-byte matmul)
PHA = 4                      # n-strips accumulated concurrently at startup

_cache = {}


def _build_nc(Kd=K, Nd=N, Md=M, compute_dt_name="float32r"):
    import concourse.bacc as bacc
    import concourse.mybir as mybir
    import concourse.tile as tile

    kt_n = Kd // P
    nt_n = Nd // P
    nmc = Md // MC
    pha = min(PHA, nt_n)
    kh = max(kt_n // 4, 1)       # k-tiles per W sub-tile
    nwh = (kt_n + kh - 1) // kh  # sub-tiles per strip
    n_oc = 4                     # drain chunks per strip

    cdt = getattr(mybir.dt, compute_dt_name)
    f32 = mybir.dt.float32

    nc = bacc.Bacc("TRN2", target_bir_lowering=False, debug=False)

    xt_d = nc.dram_tensor("xt", [Kd, Md], cdt, kind="ExternalInput").ap()
    wt_d = nc.dram_tensor("wt", [nt_n, P, kt_n * P], cdt,
                          kind="ExternalInput").ap()
    bt_d = nc.dram_tensor("bt", [P, nt_n], f32, kind="ExternalInput").ap()
    ot_d = nc.dram_tensor("ot", [Nd, Md], f32, kind="ExternalOutput").ap()

    with tile.TileContext(nc) as tc:
        with (
            tc.tile_pool(name="xpool", bufs=kt_n) as xpool,
            tc.tile_pool(name="wpool", bufs=min(4 * nwh, nwh * nt_n))
                as wpool,
            tc.tile_pool(name="psum", bufs=min(4, nt_n),
                         space="PSUM") as psumpool,
            tc.tile_pool(name="opool", bufs=4) as opool,
            tc.tile_pool(name="bpool", bufs=1) as bpool,
        ):
            w_tiles = {}   # (nt, half) -> tile

            def load_wh(nt, h):
                w_sb = wpool.tile([P, kh * P], cdt, tag="w",
                                  name=f"w{nt}_{h}")
                nc.sync.dma_start(
                    w_sb[:], wt_d[nt][:, h * kh * P:(h + 1) * kh * P])
                w_tiles[(nt, h)] = w_sb

            def mm(ps, nt, kt, mc):
                w_sb = w_tiles[(nt, kt // kh)]
                nc.tensor.matmul(
                    ps[:, mc * MC:(mc + 1) * MC],
                    w_sb[:, (kt % kh) * P:(kt % kh + 1) * P],
                    x_tiles[kt][:, mc * MC:(mc + 1) * MC],
                    start=(kt == 0),
                    stop=(kt == kt_n - 1),
                )

            def drain(nt, ps, chunks=n_oc, dma_engine=None):
                # chunked, alternating ScalarE/VectorE so the PSUM drain is
                # 2x wide; out DMA off the critical queues
                dma_engine = dma_engine or nc.gpsimd
                h = Md // chunks
                for i in range(chunks):
                    sl = slice(i * h, (i + 1) * h)
                    out_sb = opool.tile([P, h], f32, tag="o")
                    if i % 2 == 0:
                        nc.scalar.activation(
                            out_sb[:], ps[:, sl],
                            mybir.ActivationFunctionType.Identity,
                            bias=bias_sb[:, nt:nt + 1],
                        )
                    else:
                        nc.vector.tensor_scalar_add(
                            out_sb[:], ps[:, sl], bias_sb[:, nt:nt + 1])
                    dma_engine.dma_start(ot_d[nt * P:(nt + 1) * P, sl],
                                         out_sb[:])

            def load_w_strip(nt):
                for q in range(nwh):
                    load_wh(nt, q)

            def release_w(nt):
                for q in range(nwh):
                    del w_tiles[(nt, q)]

            # Startup issue order: interleave the phase-A W halves with the
            # first x k-tiles so the PE can start at the first (w,x) pair.
            x_tiles = []

            def load_next_x(n=1):
                for _ in range(n):
                    kt = len(x_tiles)
                    if kt >= kt_n:
                        return
                    x_sb = xpool.tile([P, Md], cdt, tag="x", name=f"x{kt}")
                    nc.sync.dma_start(x_sb[:], xt_d[kt * P:(kt + 1) * P, :])
                    x_tiles.append(x_sb)

            # PE warm-up: dummy matmuls on zeroed scratch un-throttle the
            # HAM clock gate (~3.4us of sustained activity) while the first
            # DMAs are still in flight, so real matmuls start at 2.4 GHz.
            warm_sb = bpool.tile([P, 256], f32, name="warm")
            nc.vector.memset(warm_sb[:], 0.0)
            warm_ps = psumpool.tile([P, 256], f32, tag="ps", name="warmps")
            for _ in range(8):
                nc.tensor.matmul(warm_ps[:], warm_sb[:, 0:P], warm_sb[:],
                                 start=True, stop=True)

            # Issue order on the sync queue follows phase A's need order:
            # w_s half-0 just before the x tiles strip s will chew first.
            load_wh(0, 0)
            load_next_x(2)
            for s in range(1, pha):
                load_wh(s, 0)
                load_next_x(2)
            bias_sb = bpool.tile([P, nt_n], f32)
            nc.sync.dma_start(bias_sb[:], bt_d[:])
            load_next_x(4)
            for q in range(1, nwh):
                for s in range(pha):
                    load_wh(s, q)
                    load_next_x(1)
            load_next_x(kt_n)

            # Phase A: strips 0..pha-1 accumulate while x streams. Walk
            # k-blocks with the strip loop outside the block's k-loop so the
            # first strip only needs its own W half plus the first x tiles.
            pss = [psumpool.tile([P, Md], f32, tag="ps", name=f"ps{s}")
                   for s in range(pha)]
            kb_sz = min(8, kt_n)
            for kb in range(0, kt_n, kb_sz):
                for s in range(pha):
                    for kt in range(kb, kb + kb_sz):
                        for mc in range(nmc):
                            mm(pss[s], s, kt, mc)
            # Prefetch the next W strips as slots free up.
            for nt in range(pha, min(pha + 2, nt_n)):
                load_w_strip(nt)
            for s in range(pha):
                drain(s, pss[s])
                release_w(s)

            # Phase B: one strip at a time.
            for nt in range(pha, nt_n):
                if nt + 2 < nt_n:
                    load_w_strip(nt + 2)
                ps = psumpool.tile([P, Md], f32, tag="ps")
                for kt in range(kt_n):
                    for mc in range(nmc):
                        mm(ps, nt, kt, mc)
                drain(nt, ps, dma_engine=nc.sync if nt == nt_n - 1 else None)
                release_w(nt)

    nc.compile()
    return nc


def _get_nc():
    if "nc" not in _cache:
        _cache["nc"] = _build_nc()
    return _cache["nc"]


def _pack_w(weight, Nd=N, Kd=K):
    nt_n, kt_n = Nd // P, Kd // P
    # packed[nt, p, kt, nl] = weight[nt*P + nl, kt*P + p]
    wr = weight.reshape(nt_n, P, kt_n, P)          # [nt, nl, kt, p]
    return np.ascontiguousarray(
        wr.transpose(0, 3, 2, 1)).reshape(nt_n, P, kt_n * P)


def _prep_inputs(x, weight, b):
    x = np.ascontiguousarray(x, dtype=np.float32)
    weight = np.ascontiguousarray(weight, dtype=np.float32)
    b = np.ascontiguousarray(b, dtype=np.float32)

    xt = np.ascontiguousarray(x.reshape(M_TOTAL, K).T)       # [K, M_TOTAL]
    wt = _pack_w(weight)                                     # [NT, P, KT*P]
    bt = np.ascontiguousarray(b.reshape(NT, P).T)            # [P, NT]

    in_maps = []
    for c in range(N_CORES):
        in_maps.append({
            "xt": np.ascontiguousarray(xt[:, c * M:(c + 1) * M]),
            "wt": wt,
            "bt": bt,
        })
    return in_maps


def run(x, weight, b, trace=False, **trace_kwargs):
    from concourse.bass_utils import run_bass_kernel_spmd

    nc = _get_nc()
    in_maps = _prep_inputs(x, weight, b)
    res = run_bass_kernel_spmd(
        nc, in_maps, list(range(N_CORES)), trace=trace, **trace_kwargs
    )

    out = np.empty((M_TOTAL, N), dtype=np.float32)
    for c in range(N_CORES):
        out[c * M:(c + 1) * M, :] = res.results[c]["ot"].T
    return out.reshape(B, S, N), res


def kernel(x, weight, b, tile_size=None):
    out, _ = run(x, weight, b)
    return out


# revision 26
# speedup vs baseline: 1.0139x; 1.0086x over previous
"""Batch-invariant linear (out = x @ W.T + b) on 8 TRN2 NeuronCores.

Strategy: data-parallel over the 8192 (batch*seq) rows — 1024 rows/core.
Per core we compute out^T[n, m] so the contraction dim K lands on SBUF
partitions for both operands with no on-device transposes:
  - host pre-transposes x -> xT [K, M] and packs W into per-n-strip
    contiguous blocks [NT, 128, KT*128] (layout prep only),
  - stationary operand = WT tile [128k, 128n], moving = xT [128k, 512m],
  - f32r (TF32) matmuls at full PE rate, fp32 PSUM accumulation over the
    32 k-tiles in increasing k order (deterministic, batch-invariant),
  - startup: 4 n-strips accumulate k-major (filling all 8 PSUM banks)
    so the PE stays busy while the 16.8 MB x shard streams in,
  - bias added on ScalarE during PSUM->SBUF drain, DMA out^T shard out.
Host gathers the 8 out^T shards and transposes back.
"""

import numpy as np

N_CORES = 8
B, S, K, N = 4, 2048, 4096, 4096
M_TOTAL = B * S              # 8192 rows
M = M_TOTAL // N_CORES       # 1024 rows per core
P = 128                      # partitions
KT = K // P                  # 32 k-tiles
NT = N // P                  # 32 n-tiles (out^T partition tiles)
MC = 512                     # moving chunk (max free dim for 4-byte matmul)
PHA = 4                      # n-strips accumulated concurrently at startup

_cache = {}


def _build_nc(Kd=K, Nd=N, Md=M, compute_dt_name="float32r"):
    import concourse.bacc as bacc
    import concourse.mybir as mybir
    import concourse.tile as tile

    kt_n = Kd // P
    nt_n = Nd // P
    nmc = Md // MC
    pha = min(PHA, nt_n)
    kh = max(kt_n // 4, 1)       # k-tiles per W sub-tile
    nwh = (kt_n + kh - 1) // kh  # sub-tiles per strip
    n_oc = 4                     # drain chunks per strip

    cdt = getattr(mybir.dt, compute_dt_name)
    f32 = mybir.dt.float32

    nc = bacc.Bacc("TRN2", target_bir_lowering=False, debug=False)

    xt_d = nc.dram_tensor("xt", [Kd, Md], cdt, kind="ExternalInput").ap()
    wt_d = nc.dram_tensor("wt", [nt_n, P, kt_n * P], cdt,
                          kind="ExternalInput").ap()
    bt_d = nc.dram_tensor("bt", [P, nt_n], f32, kind="ExternalInput").ap()
    ot_d = nc.dram_tensor("ot", [Nd, Md], f32, kind="ExternalOutput").ap()

    with tile.TileContext(nc) as tc:
        with (
            tc.tile_pool(name="xpool", bufs=kt_n) as xpool,
            tc.tile_pool(name="wpool", bufs=min(4 * nwh, nwh * nt_n))
                as wpool,
            tc.tile_pool(name="psum", bufs=min(4, nt_n),
                         space="PSUM") as psumpool,
            tc.tile_pool(name="opool", bufs=4) as opool,
            tc.tile_pool(name="bpool", bufs=1) as bpool,
        ):
            w_tiles = {}   # (nt, half) -> tile

            def load_wh(nt, h):
                w_sb = wpool.tile([P, kh * P], cdt, tag="w",
                                  name=f"w{nt}_{h}")
                nc.sync.dma_start(
                    w_sb[:], wt_d[nt][:, h * kh * P:(h + 1) * kh * P])
                w_tiles[(nt, h)] = w_sb

            def mm(ps, nt, kt, mc):
                w_sb = w_tiles[(nt, kt // kh)]
                nc.tensor.matmul(
                    ps[:, mc * MC:(mc + 1) * MC],
                    w_sb[:, (kt % kh) * P:(kt % kh + 1) * P],
                    x_tiles[kt][:, mc * MC:(mc + 1) * MC],
                    start=(kt == 0),
                    stop=(kt == kt_n - 1),
                )

            def drain(nt, ps, chunks=n_oc, dma_engine=None):
                # chunked, alternating ScalarE/VectorE so the PSUM drain is
                # 2x wide; out DMA off the critical queues
                dma_engine = dma_engine or nc.gpsimd
                h = Md // chunks
                for i in range(chunks):
                    sl = slice(i * h, (i + 1) * h)
                    out_sb = opool.tile([P, h], f32, tag="o")
                    if i % 2 == 0:
                        nc.scalar.activation(
                            out_sb[:], ps[:, sl],
                            mybir.ActivationFunctionType.Identity,
                            bias=bias_sb[:, nt:nt + 1],
                        )
                    else:
                        nc.vector.tensor_scalar_add(
                            out_sb[:], ps[:, sl], bias_sb[:, nt:nt + 1])
                    dma_engine.dma_start(ot_d[nt * P:(nt + 1) * P, sl],
                                         out_sb[:])

            def load_w_strip(nt):
                for q in range(nwh):
                    load_wh(nt, q)

            def release_w(nt):
                for q in range(nwh):
                    del w_tiles[(nt, q)]

            # Startup issue order: interleave the phase-A W halves with the
            # first x k-tiles so the PE can start at the first (w,x) pair.
            x_tiles = []

            def load_next_x(n=1):
                for _ in range(n):
                    kt = len(x_tiles)
                    if kt >= kt_n:
                        return
                    x_sb = xpool.tile([P, Md], cdt, tag="x", name=f"x{kt}")
                    nc.sync.dma_start(x_sb[:], xt_d[kt * P:(kt + 1) * P, :])
                    x_tiles.append(x_sb)

            # PE warm-up: dummy matmuls on zeroed scratch un-throttle the
            # HAM clock gate (~3.4us of sustained activity) while the first
            # DMAs are still in flight, so real matmuls start at 2.4 GHz.
            warm_sb = bpool.tile([P, 256], f32, name="warm")
            nc.vector.memset(warm_sb[:], 0.0)
            warm_ps = psumpool.tile([P, 256], f32, tag="ps", name="warmps")
            for _ in range(8):
                nc.tensor.matmul(warm_ps[:], warm_sb[:, 0:P], warm_sb[:],
                                 start=True, stop=True)

            # Issue order on the sync queue follows phase A's need order:
            # w_s half-0 just before the x tiles strip s will chew first.
            load_wh(0, 0)
            load_next_x(2)
            for s in range(1, pha):
                load_wh(s, 0)
                load_next_x(2)
            bias_sb = bpool.tile([P, nt_n], f32)
            nc.sync.dma_start(bias_sb[:], bt_d[:])
            load_next_x(4)
            for q in range(1, nwh):
                for s in range(pha):
                    load_wh(s, q)
                    load_next_x(1)
            load_next_x(kt_n)

            # Phase A: strips 0..pha-1 accumulate while x streams. Walk
            # k-blocks with the strip loop outside the block's k-loop so the
            # first strip only needs its own W half plus the first x tiles.
            pss = [psumpool.tile([P, Md], f32, tag="ps", name=f"ps{s}")
                   for s in range(pha)]
            # small k-blocks keep any x-pacing stall under the ~3.4us HAM
            # idle window, so the PE clock never re-throttles mid-startup
            kb_sz = min(4, kt_n)
            for kb in range(0, kt_n, kb_sz):
                for s in range(pha):
                    for kt in range(kb, kb + kb_sz):
                        for mc in range(nmc):
                            mm(pss[s], s, kt, mc)
            # Prefetch the next W strips as slots free up.
            for nt in range(pha, min(pha + 2, nt_n)):
                load_w_strip(nt)
            for s in range(pha):
                drain(s, pss[s])
                release_w(s)

            # Phase B: one strip at a time.
            for nt in range(pha, nt_n):
                if nt + 2 < nt_n:
                    load_w_strip(nt + 2)
                ps = psumpool.tile([P, Md], f32, tag="ps")
                for kt in range(kt_n):
                    for mc in range(nmc):
                        mm(ps, nt, kt, mc)
                drain(nt, ps, dma_engine=nc.sync if nt == nt_n - 1 else None)
                release_w(nt)

    nc.compile()
    return nc


def _get_nc():
    if "nc" not in _cache:
        _cache["nc"] = _build_nc()
    return _cache["nc"]


def _pack_w(weight, Nd=N, Kd=K):
    nt_n, kt_n = Nd // P, Kd // P
    # packed[nt, p, kt, nl] = weight[nt*P + nl, kt*P + p]
    wr = weight.reshape(nt_n, P, kt_n, P)          # [nt, nl, kt, p]
    return np.ascontiguousarray(
        wr.transpose(0, 3, 2, 1)).reshape(nt_n, P, kt_n * P)


def _prep_inputs(x, weight, b):
    x = np.ascontiguousarray(x, dtype=np.float32)
    weight = np.ascontiguousarray(weight, dtype=np.float32)
    b = np.ascontiguousarray(b, dtype=np.float32)

    xt = np.ascontiguousarray(x.reshape(M_TOTAL, K).T)       # [K, M_TOTAL]
    wt = _pack_w(weight)                                     # [NT, P, KT*P]
    bt = np.ascontiguousarray(b.reshape(NT, P).T)            # [P, NT]

    in_maps = []
    for c in range(N_CORES):
        in_maps.append({
            "xt": np.ascontiguousarray(xt[:, c * M:(c + 1) * M]),
            "wt": wt,
            "bt": bt,
        })
    return in_maps


def run(x, weight, b, trace=False, **trace_kwargs):
    from concourse.bass_utils import run_bass_kernel_spmd

    nc = _get_nc()
    in_maps = _prep_inputs(x, weight, b)
    res = run_bass_kernel_spmd(
        nc, in_maps, list(range(N_CORES)), trace=trace, **trace_kwargs
    )

    out = np.empty((M_TOTAL, N), dtype=np.float32)
    for c in range(N_CORES):
        out[c * M:(c + 1) * M, :] = res.results[c]["ot"].T
    return out.reshape(B, S, N), res


def kernel(x, weight, b, tile_size=None):
    out, _ = run(x, weight, b)
    return out
